# revision 1
# baseline (speedup 1.0000x reference)
"""DGCNN inference kernel for Trainium2 (Bass/Tile), 8 NeuronCores.

Strategy: pure data-parallel — one point cloud per core (B=8).

Per-core algorithm (all SBUF-resident, channels-major layout xT (C, N)):
  For each of 4 edge-conv layers:
    - KNN selection scores d(n,j) = x_n.x_j - s_j/2 (row-constant terms
      dropped; ranking-equivalent to the reference's -|x_n - x_j|^2).
      Computed per 128-point tile on the PE with bf16 hi/lo split
      (3 passes, ~1e-5 rel accuracy) into PSUM, staged to SBUF.
    - Exact top-20 per row via two-level DVE selection:
        stage A: nc.vector.max (top-8) + max_index per group of 128
        stage B: 3 rounds of max/max_index/match_replace over the 256
                 candidates (exact jax.lax.top_k tie semantics).
      Candidate-position -> global-index mapping resolved with a small
      indirect DMA gather through a DRAM scratch buffer.
    - Since BN scale > 0 and LeakyReLU is monotone, max over neighbors
      commutes with the affine edge transform:
        out(n) = lrelu(a*(max_j tT(:,j) + cT(:,n)) + b)
      tT = Wd @ xT precomputed (O, N); neighbor columns gathered with
      gpsimd.indirect_copy; max over k=20 by one strided tensor_reduce.
  Then the global 1x1 conv (W5), max+mean pool over N, and the small MLP
  head, with all BatchNorms folded host-side into per-channel affines.
"""

import numpy as np
import sys

sys.path.insert(0, "/opt/trn_rl_repo")

import concourse.bass as bass
import concourse.bacc as bacc
import concourse.tile as tile
from concourse import mybir
from concourse.bass_utils import run_bass_kernel_spmd

FP32 = mybir.dt.float32
BF16 = mybir.dt.bfloat16
U16 = mybir.dt.uint16
U32 = mybir.dt.uint32

AF = mybir.ActivationFunctionType
ALU = mybir.AluOpType
AX = mybir.AxisListType

N = 4096
K = 20
NTILES = N // 128
GS = 256  # stage-A group size
NGROUPS = N // GS  # 16 groups * 8 = 128 candidates
NCAND = NGROUPS * 8
EPS = 1e-5
NEG_BIG = -3.0e38

# edge conv layer dims (Cin, Cout)
LAYERS = [(6, 64), (64, 64), (64, 128), (128, 256)]


def _split_hl(x):
    import ml_dtypes

    hi = x.astype(ml_dtypes.bfloat16)
    lo = (x.astype(np.float32) - hi.astype(np.float32)).astype(ml_dtypes.bfloat16)
    return hi, lo


def _bn_affine(g, b, rm, rv):
    a = g / np.sqrt(rv + EPS)
    return a.astype(np.float32), (b - a * rm).astype(np.float32)


class HLFeat:
    """Hi/lo bf16 feature storage for C channels x N points. Hi and lo occupy
    the SAME row range (base partition) in two separate tiles, so every
    (hi,lo) matmul operand pair shares its base partition (PE requires equal
    bases in {0,32,64}). chunks = [(hi_tile, lo_tile, base_row), ...] one per
    128 channels."""

    def __init__(self, C, chunks):
        self.C = C
        self.chunks = chunks

    @staticmethod
    def _sl(t, base, c0, c1, csl):
        return t[base + c0 : base + c1, csl]

    def hi(self, c0, c1, csl=slice(None)):
        ti = c0 // 128
        assert c1 - c0 <= 128 and (c1 - 1) // 128 == ti, (c0, c1)
        th, tl, b = self.chunks[ti]
        return self._sl(th, b, c0 - 128 * ti, c1 - 128 * ti, csl)

    def lo(self, c0, c1, csl=slice(None)):
        ti = c0 // 128
        assert c1 - c0 <= 128 and (c1 - 1) // 128 == ti, (c0, c1)
        th, tl, b = self.chunks[ti]
        return self._sl(tl, b, c0 - 128 * ti, c1 - 128 * ti, csl)


class Prog:
    """Builds the bass program once; holds tensor handles + host-const inputs."""

    def __init__(self):
        self.nc = bacc.Bacc(
            "TRN2", target_bir_lowering=False, debug=False, num_devices=8
        )
        self.consts = {}  # name -> np array (same for all cores)
        self.build()

    def inp(self, name, shape, dtype):
        return self.nc.declare_dram_parameter(name, list(shape), dtype, isOutput=False)

    def const(self, name, arr, dtype):
        h = self.nc.declare_dram_parameter(name, list(arr.shape), dtype, isOutput=False)
        self.consts[name] = arr
        return h

    def build(self):
        nc = self.nc
        x_in = self.inp("x", (6, N), FP32)
        self.out = nc.declare_dram_parameter("out", [2, 1], FP32, isOutput=True)

        wh = {}
        for li, (C, O) in enumerate(LAYERS):
            for nm, shape in [
                (f"wdh{li}", (C, O)),
                (f"wdl{li}", (C, O)),
                (f"cwh{li}", (C, O)),
                (f"cwl{li}", (C, O)),
            ]:
                wh[nm] = self.inp(nm, shape, BF16)
            for nm in ["a06", "b06", "a04", "b04"]:
                wh[f"{nm}_{li}"] = self.inp(f"{nm}_{li}", (O, 1), FP32)
        self.w5chunks = [(1, 0, 64), (2, 0, 64), (3, 0, 128), (4, 0, 128), (4, 128, 128)]
        for i, (fi, c0, cc) in enumerate(self.w5chunks):
            wh[f"w5h_{i}"] = self.inp(f"w5h_{i}", (cc, 1024), BF16)
            wh[f"w5l_{i}"] = self.inp(f"w5l_{i}", (cc, 1024), BF16)
        for nm in ["a06_5", "b06_5", "a04_5", "b04_5"]:
            wh[nm] = self.inp(nm, (1024, 1), FP32)
        wh["l1t"] = self.inp("l1t", (2048, 512), FP32)
        for nm in ["a06_l1", "b06_l1", "a04_l1", "b04_l1"]:
            wh[nm] = self.inp(nm, (512, 1), FP32)
        wh["l2t"] = self.inp("l2t", (512, 256), FP32)
        for nm in ["a06_l2", "b06_l2", "a04_l2", "b04_l2"]:
            wh[nm] = self.inp(nm, (256, 1), FP32)
        wh["l3t"] = self.inp("l3t", (256, 2), FP32)
        wh["l3b"] = self.inp("l3b", (2, 1), FP32)
        self.wh = wh

        off = np.zeros((128, NCAND), np.uint16)
        for g in range(NGROUPS):
            off[:, 8 * g : 8 * g + 8] = GS * g
        OFF = self.const("OFFC", off, U16)
        pcb = np.broadcast_to(
            (np.arange(128, dtype=np.uint32) * NCAND)[:, None], (128, K)
        ).copy()
        PCB = self.const("PCB", pcb, U32)

        ldram = [nc.dram_tensor(f"lscr{i}", [128 * NCAND, 1], U16) for i in range(8)]
        sdram = [
            (
                nc.dram_tensor(f"sscrp{i}", [128 * K], U16),
                nc.dram_tensor(f"sscrq{i}", [128 * K], U16),
            )
            for i in range(8)
        ]

        with tile.TileContext(nc) as tc:
            self._emit(tc, x_in, wh, OFF, PCB, ldram, sdram)
        nc.compile()

    # ------------------------------------------------------------------
    def _emit(self, tc, x_in, wh, OFF, PCB, ldram, sdram):
        nc = self.nc
        from contextlib import ExitStack

        with ExitStack() as ctx:
            pers = ctx.enter_context(tc.tile_pool(name="pers", bufs=1))
            x1h = pers.tile([64, N], BF16, name="x1h", tag="x1h")
            x1l = pers.tile([64, N], BF16, name="x1l", tag="x1l")
            x2h = pers.tile([64, N], BF16, name="x2h", tag="x2h")
            x2l = pers.tile([64, N], BF16, name="x2l", tag="x2l")
            x3h = pers.tile([128, N], BF16, name="x3h", tag="x3h")
            x3l = pers.tile([128, N], BF16, name="x3l", tag="x3l")
            x4h0 = pers.tile([128, N], BF16, name="x4h0", tag="x4h0")
            x4h1 = pers.tile([128, N], BF16, name="x4h1", tag="x4h1")
            x4l0 = pers.tile([128, N], BF16, name="x4l0", tag="x4l0")
            x4l1 = pers.tile([128, N], BF16, name="x4l1", tag="x4l1")
            feat = [
                None,  # x0 installed below (scoped pool)
                HLFeat(64, [(x1h, x1l, 0)]),
                HLFeat(64, [(x2h, x2l, 0)]),
                HLFeat(128, [(x3h, x3l, 0)]),
                HLFeat(256, [(x4h0, x4l0, 0), (x4h1, x4l1, 0)]),
            ]
            srow = pers.tile([1, N], FP32, name="srow", tag="srow")

            consts = ctx.enter_context(tc.tile_pool(name="consts", bufs=1))
            offt = consts.tile([128, NCAND], U16, name="offt")
            nc.sync.dma_start(offt[:], OFF[:], single_packet=True)
            pcbt = consts.tile([128, K], U32, name="pcbt")
            nc.sync.dma_start(pcbt[:], PCB[:], single_packet=True)
            onescol = consts.tile([128, 1], FP32, name="onescol")
            nc.vector.memset(onescol[:], 1.0)
            mhalf = consts.tile([1, 128], FP32, name="mhalf")
            nc.vector.memset(mhalf[:], -0.5)

            # edge-layer weights
            wpool = ctx.enter_context(tc.tile_pool(name="wpool", bufs=1))
            wt = {}
            for li, (C, O) in enumerate(LAYERS):
                for nm in [f"wdh{li}", f"wdl{li}", f"cwh{li}", f"cwl{li}"]:
                    wt[nm] = wpool.tile(list(wh[nm].shape), BF16, name=nm, tag=nm)
                    nc.sync.dma_start(wt[nm][:], wh[nm][:], single_packet=True)
                for nm in [f"a06_{li}", f"b06_{li}", f"a04_{li}", f"b04_{li}"]:
                    wt[nm] = [
                        wpool.tile(
                            [min(128, O - 128 * i), 1], FP32,
                            name=f"{nm}_o{i}", tag=f"{nm}_o{i}",
                        )
                        for i in range((O + 127) // 128)
                    ]
                    for i in range((O + 127) // 128):
                        nc.sync.dma_start(
                            wt[nm][i][:], wh[nm][128 * i : min(128 * (i + 1), O), :],
                            single_packet=True,
                        )

            xp_ctx = ExitStack()
            xin_pool = xp_ctx.enter_context(tc.tile_pool(name="xin", bufs=1))
            xp = xp_ctx.enter_context(tc.tile_pool(name="x0p", bufs=1))
            x0h = xp.tile([6, N], BF16, name="x0h", tag="x0h")
            x0l = xp.tile([6, N], BF16, name="x0l", tag="x0l")
            feat[0] = HLFeat(6, [(x0h, x0l, 0)])
            x0 = xin_pool.tile([6, N], FP32, name="x0")
            nc.sync.dma_start(x0[:], x_in[:], single_packet=True)
            xh32 = xin_pool.tile([6, N], FP32, name="xh32")
            nc.scalar.copy(feat[0].hi(0, 6), x0[:])  # cast fp32->bf16
            nc.scalar.copy(xh32[:], feat[0].hi(0, 6))  # back to fp32
            nc.vector.tensor_sub(feat[0].lo(0, 6), x0[:], xh32[:])

            for li, (C, O) in enumerate(LAYERS):
                self._edge_layer(
                    tc, li, C, O, feat[li], feat[li + 1], srow, wt,
                    offt, pcbt, onescol, mhalf, ldram, sdram,
                )
                if li == 0:
                    xp_ctx.close()

            self._head(tc, feat, wh)

    # ------------------------------------------------------------------
    def _edge_layer(
        self, tc, li, C, O, fin, fout, srow, wt, offt, pcbt, onescol, mhalf, ldram,
        sdram,
    ):
        nc = self.nc
        from contextlib import ExitStack

        nct = (C + 127) // 128
        not_ = (O + 127) // 128

        def cr(ci):
            c0 = 128 * ci
            return c0, min(128 * (ci + 1), C)

        with ExitStack() as octx:
          tpool = octx.enter_context(tc.tile_pool(name=f"tT{li}", bufs=1))
          with ExitStack() as lctx:
            # ---- layer setup: s row and tT ----
            sp = lctx.enter_context(tc.tile_pool(name=f"sq{li}", bufs=1))
            spp = lctx.enter_context(tc.tile_pool(name=f"spp{li}", bufs=2, space="PSUM"))
            xsq_tiles = []
            for ci in range(nct):
                c0, c1 = cr(ci)
                cc = c1 - c0
                u = sp.tile([cc, N], FP32, name="xsqu", tag="xsqu")
                nc.vector.tensor_scalar_mul(u[:], fin.lo(c0, c1), 2.0)
                nc.vector.tensor_add(u[:], u[:], fin.hi(c0, c1))
                xsq = sp.tile([cc, N], FP32, name="xsq", tag=f"xsq{ci}")
                nc.vector.tensor_mul(xsq[:], u[:], fin.hi(c0, c1))
                xsq_tiles.append((xsq, cc))
            for ch in range(N // 512):
                ps = spp.tile([1, 512], FP32, name="spsum", tag="spsum")
                for ti, (xsq, cc) in enumerate(xsq_tiles):
                    nc.tensor.matmul(
                        ps[:],
                        onescol[0:cc, :],
                        xsq[:, 512 * ch : 512 * (ch + 1)],
                        start=(ti == 0),
                        stop=(ti == len(xsq_tiles) - 1),
                    )
                nc.scalar.copy(srow[:, 512 * ch : 512 * (ch + 1)], ps[:])

            # tT (O, N) = Wd @ x  (hi/lo 3-pass)
            tT = [
                tpool.tile([min(128, O - 128 * oi), N], FP32, name=f"tT{oi}", tag=f"tT{oi}")
                for oi in range(not_)
            ]
            tpp = lctx.enter_context(tc.tile_pool(name=f"tpp{li}", bufs=2, space="PSUM"))
            for oi in range(not_):
                o0, o1 = 128 * oi, min(128 * (oi + 1), O)
                for ch in range(N // 512):
                    pt = tpp.tile([o1 - o0, 512], FP32, name="tpsum", tag="tpsum")
                    sl = slice(512 * ch, 512 * (ch + 1))
                    mms = []
                    for ci in range(nct):
                        c0, c1 = cr(ci)
                        wdh = wt[f"wdh{li}"][c0:c1, o0:o1]
                        wdl = wt[f"wdl{li}"][c0:c1, o0:o1]
                        mms += [
                            (wdh, fin.hi(c0, c1, sl)),
                            (wdl, fin.hi(c0, c1, sl)),
                            (wdh, fin.lo(c0, c1, sl)),
                        ]
                    for mi, (lhsT, rhs) in enumerate(mms):
                        nc.tensor.matmul(
                            pt[:], lhsT, rhs,
                            start=(mi == 0), stop=(mi == len(mms) - 1),
                        )
                    nc.scalar.copy(tT[oi][:, sl], pt[:])

          # ---- per-tile selection + gather loop ----
          with ExitStack() as tctx:
              dpp = tctx.enter_context(tc.tile_pool(name=f"dp{li}", bufs=3, space="PSUM"))
              cpp = tctx.enter_context(tc.tile_pool(name=f"cp{li}", bufs=1, space="PSUM"))
              dsb = tctx.enter_context(tc.tile_pool(name=f"dsb{li}", bufs=3))
              sel = tctx.enter_context(tc.tile_pool(name=f"sel{li}", bufs=2))
              gat = tctx.enter_context(tc.tile_pool(name=f"gat{li}", bufs=2))

              for ti in range(NTILES):
                  tsl = slice(128 * ti, 128 * (ti + 1))
                  cand = sel.tile([128, NCAND], FP32, name="cand", tag="cand")
                  lidx = sel.tile([128, NCAND], U16, name="lidx", tag="lidx")
                  for q in range(4):
                      pq = dpp.tile([128, 1024], FP32, name="dq", tag="dq")
                      for ch in range(2):
                          qsl = slice(1024 * q + 512 * ch, 1024 * q + 512 * (ch + 1))
                          psl = slice(512 * ch, 512 * (ch + 1))
                          mms = []
                          for ci in range(nct):
                              c0, c1 = cr(ci)
                              mms += [
                                  (fin.hi(c0, c1, tsl), fin.hi(c0, c1, qsl)),
                                  (fin.lo(c0, c1, tsl), fin.hi(c0, c1, qsl)),
                                  (fin.hi(c0, c1, tsl), fin.lo(c0, c1, qsl)),
                              ]
                          mms.append((mhalf[:, 0:128], srow[:, qsl]))
                          for mi, (lhsT, rhs) in enumerate(mms):
                              nc.tensor.matmul(
                                  pq[:, psl], lhsT, rhs,
                                  start=(mi == 0), stop=(mi == len(mms) - 1),
                              )
                      Dq = dsb.tile([128, 1024], FP32, name="Dq", tag="Dq")
                      nc.scalar.copy(Dq[:], pq[:])
                      # stage A on this chunk (4 groups of 256)
                      for g in range(4):
                          gg = 4 * q + g
                          osl = slice(8 * gg, 8 * gg + 8)
                          gsl = slice(GS * g, GS * (g + 1))
                          nc.vector.max(cand[:, osl], Dq[:, gsl])
                          nc.vector.max_index(lidx[:, osl], cand[:, osl], Dq[:, gsl])
                  nc.vector.tensor_add(lidx[:], lidx[:], offt[:])
                  # stage B
                  posc = sel.tile([128, 24], U32, name="posc", tag="posc")
                  m8 = sel.tile([128, 8], FP32, name="m8", tag="m8")
                  for r in range(3):
                      nc.vector.max(m8[:], cand[:])
                      nc.vector.max_index(posc[:, 8 * r : 8 * r + 8], m8[:], cand[:])
                      if r < 2:
                          nc.vector.match_replace(cand[:], m8[:], cand[:], NEG_BIG)
                  idx2 = sel.tile([128, K], U32, name="idx2", tag="idx2")
                  nc.vector.tensor_add(idx2[:], posc[:, 0:K], pcbt[:])
                  ld = ldram[ti % 8]
                  nc.sync.dma_start(
                      ld[:].rearrange("(p c) one -> p (c one)", p=128), lidx[:],
                      single_packet=True,
                  )
                  sel20 = sel.tile([128, K], U16, name="sel20", tag="sel20", bufs=6)
                  # HW indirect DMA semantics: ONE offset per partition -> one
                  # DMA per neighbor slot
                  for s in range(K):
                      nc.gpsimd.indirect_dma_start(
                          sel20[:, s : s + 1],
                          None,
                          ld[:],
                          bass.IndirectOffsetOnAxis(ap=idx2[:, s : s + 1], axis=0),
                      )
                  # bounce sel20 through DRAM: plain write, reorder to a
                  # q-major layout, then one zero-step broadcast read back into
                  # the 16-partition-wrapped idx layout
                  sdp, sdq = sdram[ti % 8]
                  # build the wrapped q-major layout directly: one contiguous
                  # DMA per 16-partition block (the old single scattered
                  # transpose DMA cost ~60us of wire time per tile)
                  sdqv = sdq[:].rearrange("(q c s) -> q c s", q=16, c=8, s=K)
                  for c in range(8):
                      nc.sync.dma_start(
                          sdqv[:, c, :],
                          sel20[16 * c : 16 * c + 16, :],
                          single_packet=True,
                      )
                  widx = sel.tile([128, 8 * K], U16, name="widx", tag="widx", bufs=6)
                  sv = sdq[:].rearrange("(q f) -> q f", q=16)
                  for gq in range(8):
                      nc.sync.dma_start(
                          widx[16 * gq : 16 * gq + 16, :], sv, single_packet=True
                      )

                  for oi in range(not_):
                      o0, o1 = 128 * oi, min(128 * (oi + 1), O)
                      oo = o1 - o0
                      G = gat.tile([oo, 128 * K], FP32, name="G", tag="G")
                      nc.gpsimd.ap_gather(
                          G[:],
                          tT[oi][:],
                          widx[0:oo, :].bitcast(mybir.dt.int16),
                          channels=oo,
                          num_elems=N,
                          d=1,
                          num_idxs=128 * K,
                      )
                      M = gat.tile([oo, 128], FP32, name="M", tag=f"M{oi}")
                      nc.vector.tensor_reduce(
                          M[:],
                          G[:].rearrange("p (c s q) -> p c q s", c=8, s=K, q=16),
                          axis=AX.X,
                          op=ALU.max,
                      )
                      pc = cpp.tile([oo, 128], FP32, name="cps", tag=f"cps{oi}")
                      mms = []
                      for ci in range(nct):
                          c0, c1 = cr(ci)
                          cwh = wt[f"cwh{li}"][c0:c1, o0:o1]
                          cwl = wt[f"cwl{li}"][c0:c1, o0:o1]
                          mms += [
                              (cwh, fin.hi(c0, c1, tsl)),
                              (cwl, fin.hi(c0, c1, tsl)),
                              (cwh, fin.lo(c0, c1, tsl)),
                          ]
                      for mi, (lhsT, rhs) in enumerate(mms):
                          nc.tensor.matmul(
                              pc[:], lhsT, rhs,
                              start=(mi == 0), stop=(mi == len(mms) - 1),
                          )
                      nc.vector.tensor_add(M[:], M[:], pc[:])
                      z6 = gat.tile([oo, 128], FP32, name="z6", tag=f"z6_{oi}")
                      nc.scalar.activation(
                          z6[:], M[:], AF.Identity,
                          bias=wt[f"b06_{li}"][oi][:], scale=wt[f"a06_{li}"][oi][:],
                      )
                      z4 = gat.tile([oo, 128], FP32, name="z4", tag=f"z4_{oi}")
                      nc.scalar.activation(
                          z4[:], M[:], AF.Abs,
                          bias=wt[f"b04_{li}"][oi][:], scale=wt[f"a04_{li}"][oi][:],
                      )
                      zf = gat.tile([oo, 128], FP32, name="zf", tag=f"zf{oi}")
                      nc.vector.tensor_add(zf[:], z6[:], z4[:])
                      nc.scalar.copy(fout.hi(o0, o1, tsl), zf[:])  # hi (bf16)
                      zh32 = gat.tile([oo, 128], FP32, name="zh32", tag=f"zh32_{oi}")
                      nc.scalar.copy(zh32[:], fout.hi(o0, o1, tsl))
                      nc.vector.tensor_sub(fout.lo(o0, o1, tsl), zf[:], zh32[:])

    # ------------------------------------------------------------------
    def _head(self, tc, feat, wh):
        nc = self.nc
        from contextlib import ExitStack

        with ExitStack() as hctx:
            wp = hctx.enter_context(tc.tile_pool(name="headw", bufs=1))
            w5h, w5l = [], []
            for i, (fi, c0, cc) in enumerate(self.w5chunks):
                th = wp.tile([cc, 1024], BF16, name=f"w5h{i}", tag=f"w5h{i}")
                nc.sync.dma_start(th[:], wh[f"w5h_{i}"][:], single_packet=True)
                w5h.append(th)
                tl = wp.tile([cc, 1024], BF16, name=f"w5l{i}", tag=f"w5l{i}")
                nc.sync.dma_start(tl[:], wh[f"w5l_{i}"][:], single_packet=True)
                w5l.append(tl)
            prm = {}
            for nm in [
                "a06_5", "b06_5", "a04_5", "b04_5",
                "a06_l1", "b06_l1", "a04_l1", "b04_l1",
                "a06_l2", "b06_l2", "a04_l2", "b04_l2",
                "l3b",
            ]:
                shp = list(wh[nm].shape)
                if shp[0] > 128:
                    prm[nm] = [
                        wp.tile([128, 1], FP32, name=f"{nm}_{i}", tag=f"{nm}_{i}")
                        for i in range(shp[0] // 128)
                    ]
                    for i in range(shp[0] // 128):
                        nc.sync.dma_start(
                            prm[nm][i][:], wh[nm][128 * i : 128 * (i + 1), :],
                            single_packet=True,
                        )
                else:
                    prm[nm] = wp.tile(shp, FP32, name=nm, tag=nm)
                    nc.sync.dma_start(prm[nm][:], wh[nm][:], single_packet=True)
            l1t = [wp.tile([128, 512], FP32, name=f"l1t{i}", tag=f"l1t{i}") for i in range(16)]
            for i in range(16):
                nc.sync.dma_start(l1t[i][:], wh["l1t"][128 * i : 128 * (i + 1), :], single_packet=True)
            l2t = [wp.tile([128, 256], FP32, name=f"l2t{i}", tag=f"l2t{i}") for i in range(4)]
            for i in range(4):
                nc.sync.dma_start(l2t[i][:], wh["l2t"][128 * i : 128 * (i + 1), :], single_packet=True)
            l3t = [wp.tile([128, 2], FP32, name=f"l3t{i}", tag=f"l3t{i}") for i in range(2)]
            for i in range(2):
                nc.sync.dma_start(l3t[i][:], wh["l3t"][128 * i : 128 * (i + 1), :], single_packet=True)

            kchunks = [
                (feat[fi], c0, cc, i) for i, (fi, c0, cc) in enumerate(self.w5chunks)
            ]

            pool5 = hctx.enter_context(tc.tile_pool(name="pool5", bufs=2))
            psum5 = hctx.enter_context(tc.tile_pool(name="psum5", bufs=2, space="PSUM"))
            pooled = wp.tile([128, 16], FP32, name="pooled", tag="pooled")
            for oi in range(8):
                o0 = 128 * oi
                h = pool5.tile([128, N], FP32, name="h5", tag="h5")
                a06 = prm["a06_5"][oi // 2][o0 % 256 : o0 % 256 + 128, :] if False else None
                # param slices: prm[nm] is a list of 8x? no: (1024,1) -> 8 tiles
                for ch in range(N // 512):
                    sl = slice(512 * ch, 512 * (ch + 1))
                    pt = psum5.tile([128, 512], FP32, name="p5", tag="p5")
                    mms = []
                    for (fbuf, c0, cc, wi) in kchunks:
                        whk = w5h[wi][:, o0 : o0 + 128]
                        wlk = w5l[wi][:, o0 : o0 + 128]
                        xhc = fbuf.hi(c0, c0 + cc, sl)
                        xlc = fbuf.lo(c0, c0 + cc, sl)
                        mms += [(whk, xhc), (wlk, xhc), (whk, xlc)]
                    for mi, (lhsT, rhs) in enumerate(mms):
                        nc.tensor.matmul(
                            pt[:], lhsT, rhs,
                            start=(mi == 0), stop=(mi == len(mms) - 1),
                        )
                    z6 = pool5.tile([128, 512], FP32, name="z65", tag="z65")
                    nc.scalar.activation(
                        z6[:], pt[:], AF.Identity,
                        bias=prm["b06_5"][oi][:], scale=prm["a06_5"][oi][:],
                    )
                    z4 = pool5.tile([128, 512], FP32, name="z45", tag="z45")
                    nc.scalar.activation(
                        z4[:], pt[:], AF.Abs,
                        bias=prm["b04_5"][oi][:], scale=prm["a04_5"][oi][:],
                    )
                    nc.vector.tensor_add(h[:, sl], z6[:], z4[:])
                nc.vector.tensor_reduce(
                    pooled[:, oi : oi + 1], h[:], axis=AX.X, op=ALU.max
                )
                nc.vector.tensor_reduce(
                    pooled[:, 8 + oi : 8 + oi + 1], h[:], axis=AX.X, op=ALU.add
                )

            # ---- MLP head ----
            hp = hctx.enter_context(tc.tile_pool(name="hp", bufs=1))
            hpp = hctx.enter_context(tc.tile_pool(name="hpp", bufs=1, space="PSUM"))
            o1 = hp.tile([128, 4], FP32, name="o1", tag="o1")
            for oi in range(4):
                p1 = hpp.tile([128, 1], FP32, name="p1", tag="p1")
                for ki in range(16):
                    nc.tensor.matmul(
                        p1[:],
                        l1t[ki][:, 128 * oi : 128 * (oi + 1)],
                        pooled[:, ki : ki + 1],
                        start=(ki == 0),
                        stop=(ki == 15),
                    )
                z6 = hp.tile([128, 1], FP32, name="hz6", tag="hz6")
                nc.scalar.activation(
                    z6[:], p1[:], AF.Identity,
                    bias=prm["b06_l1"][oi][:], scale=prm["a06_l1"][oi][:],
                )
                z4 = hp.tile([128, 1], FP32, name="hz4", tag="hz4")
                nc.scalar.activation(
                    z4[:], p1[:], AF.Abs,
                    bias=prm["b04_l1"][oi][:], scale=prm["a04_l1"][oi][:],
                )
                nc.vector.tensor_add(o1[:, oi : oi + 1], z6[:], z4[:])
            o2 = hp.tile([128, 2], FP32, name="o2", tag="o2")
            for oi in range(2):
                p2 = hpp.tile([128, 1], FP32, name="p2", tag="p2")
                for ki in range(4):
                    nc.tensor.matmul(
                        p2[:],
                        l2t[ki][:, 128 * oi : 128 * (oi + 1)],
                        o1[:, ki : ki + 1],
                        start=(ki == 0),
                        stop=(ki == 3),
                    )
                z6 = hp.tile([128, 1], FP32, name="hz6b", tag="hz6b")
                nc.scalar.activation(
                    z6[:], p2[:], AF.Identity,
                    bias=prm["b06_l2"][oi][:], scale=prm["a06_l2"][oi][:],
                )
                z4 = hp.tile([128, 1], FP32, name="hz4b", tag="hz4b")
                nc.scalar.activation(
                    z4[:], p2[:], AF.Abs,
                    bias=prm["b04_l2"][oi][:], scale=prm["a04_l2"][oi][:],
                )
                nc.vector.tensor_add(o2[:, oi : oi + 1], z6[:], z4[:])
            p3 = hpp.tile([2, 1], FP32, name="p3", tag="p3")
            for ki in range(2):
                nc.tensor.matmul(
                    p3[:],
                    l3t[ki][:],
                    o2[:, ki : ki + 1],
                    start=(ki == 0),
                    stop=(ki == 1),
                )
            ofin = hp.tile([2, 1], FP32, name="ofin", tag="ofin")
            nc.scalar.activation(ofin[:], p3[:], AF.Identity, bias=prm["l3b"][:], scale=1.0)
            nc.sync.dma_start(self.out[:], ofin[:], single_packet=True)


# ----------------------------------------------------------------------
_PROG = None


def _get_prog():
    global _PROG
    if _PROG is None:
        _PROG = Prog()
    return _PROG


def _host_params(inputs):
    d = {}
    for li in range(4):
        C, O = LAYERS[li]
        W = np.asarray(inputs[f"W{li+1}"], np.float32)
        Wd, Wc = W[:, :C], W[:, C:]
        wdh, wdl = _split_hl(np.ascontiguousarray(Wd.T))
        cwh, cwl = _split_hl(np.ascontiguousarray((Wc - Wd).T))
        d[f"wdh{li}"], d[f"wdl{li}"] = wdh, wdl
        d[f"cwh{li}"], d[f"cwl{li}"] = cwh, cwl
        a, b = _bn_affine(
            np.asarray(inputs[f"g{li+1}"]), np.asarray(inputs[f"b{li+1}"]),
            np.asarray(inputs[f"m{li+1}"]), np.asarray(inputs[f"v{li+1}"]),
        )
        d[f"a06_{li}"] = np.ascontiguousarray((0.6 * a)[:, None])
        d[f"b06_{li}"] = np.ascontiguousarray((0.6 * b)[:, None])
        d[f"a04_{li}"] = np.ascontiguousarray((0.4 * a)[:, None])
        d[f"b04_{li}"] = np.ascontiguousarray((0.4 * b)[:, None])
    w5t = np.ascontiguousarray(np.asarray(inputs["W5"], np.float32).T)  # (512,1024)
    w5chunks = [64, 64, 128, 128, 128]
    k0 = 0
    for i, cc in enumerate(w5chunks):
        h, l = _split_hl(np.ascontiguousarray(w5t[k0 : k0 + cc]))
        d[f"w5h_{i}"], d[f"w5l_{i}"] = h, l
        k0 += cc
    a5, b5 = _bn_affine(
        np.asarray(inputs["g5"]), np.asarray(inputs["b5"]),
        np.asarray(inputs["m5"]), np.asarray(inputs["v5"]),
    )
    d["a06_5"] = np.ascontiguousarray((0.6 * a5)[:, None])
    d["b06_5"] = np.ascontiguousarray((0.6 * b5)[:, None])
    d["a04_5"] = np.ascontiguousarray((0.4 * a5)[:, None])
    d["b04_5"] = np.ascontiguousarray((0.4 * b5)[:, None])
    L1 = np.asarray(inputs["L1"], np.float32).copy()
    L1[:, 1024:] *= 1.0 / N
    d["l1t"] = np.ascontiguousarray(L1.T)
    al1, bl1 = _bn_affine(
        np.asarray(inputs["gl1"]), np.asarray(inputs["bl1"]),
        np.asarray(inputs["ml1"]), np.asarray(inputs["vl1"]),
    )
    d["a06_l1"] = np.ascontiguousarray((0.6 * al1)[:, None])
    d["b06_l1"] = np.ascontiguousarray((0.6 * bl1)[:, None])
    d["a04_l1"] = np.ascontiguousarray((0.4 * al1)[:, None])
    d["b04_l1"] = np.ascontiguousarray((0.4 * bl1)[:, None])
    L2 = np.asarray(inputs["L2"], np.float32)
    d["l2t"] = np.ascontiguousarray(L2.T)
    al2, bl2_ = _bn_affine(
        np.asarray(inputs["gl2"]), np.asarray(inputs["bl2"]),
        np.asarray(inputs["ml2"]), np.asarray(inputs["vl2"]),
    )
    bl2 = al2 * np.asarray(inputs["L2b"], np.float32) + bl2_
    d["a06_l2"] = np.ascontiguousarray((0.6 * al2)[:, None])
    d["b06_l2"] = np.ascontiguousarray((0.6 * bl2)[:, None])
    d["a04_l2"] = np.ascontiguousarray((0.4 * al2)[:, None])
    d["b04_l2"] = np.ascontiguousarray((0.4 * bl2)[:, None])
    d["l3t"] = np.ascontiguousarray(np.asarray(inputs["L3"], np.float32).T)
    d["l3b"] = np.ascontiguousarray(np.asarray(inputs["L3b"], np.float32)[:, None])
    return d


def _in_maps(prog, inputs):
    params = _host_params(inputs)
    x = np.asarray(inputs["x"], np.float32)
    in_maps = []
    for b in range(x.shape[0]):
        m = {"x": np.ascontiguousarray(x[b])}
        m.update(params)
        m.update(prog.consts)
        in_maps.append(m)
    return in_maps


def _post(res):
    out = np.stack(
        [np.asarray(res.results[b]["out"]).reshape(2) for b in range(8)]
    )
    return out.astype(np.float32)


def kernel(**inputs):
    prog = _get_prog()
    res = run_bass_kernel_spmd(prog.nc, _in_maps(prog, inputs), list(range(8)))
    return _post(res)


if __name__ == "__main__":
    import reference as ref

    inputs = {k: np.asarray(v) for k, v in ref.setup_inputs().items()}
    got = kernel(**inputs)
    exp = np.asarray(ref.reference(**inputs))
    err = np.abs(got - exp).max() / np.abs(exp).max()
    print("expected:\n", exp)
    print("got:\n", got)
    print("rel err:", err)



# revision 22
# speedup vs baseline: 1.2057x; 1.2057x over previous
"""DGCNN inference kernel for Trainium2 (Bass/Tile), 8 NeuronCores.

Strategy: pure data-parallel — one point cloud per core (B=8).

Per-core algorithm (all SBUF-resident, channels-major layout xT (C, N)):
  For each of 4 edge-conv layers:
    - KNN selection scores d(n,j) = x_n.x_j - s_j/2 (row-constant terms
      dropped; ranking-equivalent to the reference's -|x_n - x_j|^2).
      Computed per 128-point tile on the PE with bf16 hi/lo split
      (3 passes, ~1e-5 rel accuracy) into PSUM; the -s_j/2 column bias is
      pre-written into PSUM by the scalar engine and the matmuls
      accumulate on top (start=False), so the PE only runs bf16 passes.
    - Top-20 per row via packed-value selection: one fused DVE op
      (scalar_tensor_tensor) rewrites each PSUM chunk as
        packed = (d & 0xFFFFFF00) | (j % 256)
      i.e. the 8-bit within-group position rides in the low mantissa
      bits (4e-5 rel ranking fuzz, validated vs reference).  Stage A:
      nc.vector.max (top-8) per 256-column group -> 128 candidates.
      Stage B: 3 rounds of max/max_index/match_replace give the top-24
      packed values plus their candidate positions; the global index is
      rebuilt as (posc//8)*256 | (packed & 0xFF) with three tiny DVE
      ops.  No full-width max_index, no index side-tables, and no
      DRAM round trips.
    - Since BN scale > 0 and LeakyReLU is monotone, max over neighbors
      commutes with the affine edge transform:
        out(n) = lrelu(a*(max_j tT(:,j) + cT(:,n)) + b)
      tT = Wd @ xT precomputed (O, N); neighbor columns gathered with
      gpsimd.ap_gather (index list replicated into each 16-partition
      group by small SBUF->SBUF DMAs); max over k=20 by one strided
      tensor_reduce.
  Then the global 1x1 conv (W5), max+mean pool over N, and the small MLP
  head, with all BatchNorms folded host-side into per-channel affines.
"""

import numpy as np
import sys

sys.path.insert(0, "/opt/trn_rl_repo")

import concourse.bass as bass
import concourse.bacc as bacc
import concourse.tile as tile
from concourse import mybir
from concourse.bass_utils import run_bass_kernel_spmd

FP32 = mybir.dt.float32
BF16 = mybir.dt.bfloat16
U16 = mybir.dt.uint16
U32 = mybir.dt.uint32

AF = mybir.ActivationFunctionType
ALU = mybir.AluOpType
AX = mybir.AxisListType

N = 4096
K = 20
NTILES = N // 128
GS = 256  # stage-A group size (8-bit packed position)
NGROUPS = N // GS  # 16 groups * 8 = 128 candidates
NCAND = NGROUPS * 8
EPS = 1e-5
NEG_BIG = -3.0e38

# edge conv layer dims (Cin, Cout)
LAYERS = [(6, 64), (64, 64), (64, 128), (128, 256)]

DEBUG = False  # dump per-layer intermediates to DRAM for bring-up checks


def _split_hl(x):
    import ml_dtypes

    hi = x.astype(ml_dtypes.bfloat16)
    lo = (x.astype(np.float32) - hi.astype(np.float32)).astype(ml_dtypes.bfloat16)
    return hi, lo


def _bn_affine(g, b, rm, rv):
    a = g / np.sqrt(rv + EPS)
    return a.astype(np.float32), (b - a * rm).astype(np.float32)


class HLFeat:
    """Hi/lo bf16 feature storage for C channels x N points. Hi and lo occupy
    the SAME row range (base partition) in two separate tiles, so every
    (hi,lo) matmul operand pair shares its base partition (PE requires equal
    bases in {0,32,64}). chunks = [(hi_tile, lo_tile, base_row), ...] one per
    128 channels."""

    def __init__(self, C, chunks):
        self.C = C
        self.chunks = chunks

    @staticmethod
    def _sl(t, base, c0, c1, csl):
        return t[base + c0 : base + c1, csl]

    def hi(self, c0, c1, csl=slice(None)):
        ti = c0 // 128
        assert c1 - c0 <= 128 and (c1 - 1) // 128 == ti, (c0, c1)
        th, tl, b = self.chunks[ti]
        return self._sl(th, b, c0 - 128 * ti, c1 - 128 * ti, csl)

    def lo(self, c0, c1, csl=slice(None)):
        ti = c0 // 128
        assert c1 - c0 <= 128 and (c1 - 1) // 128 == ti, (c0, c1)
        th, tl, b = self.chunks[ti]
        return self._sl(tl, b, c0 - 128 * ti, c1 - 128 * ti, csl)


class Prog:
    """Builds the bass program once; holds tensor handles + host-const inputs."""

    def __init__(self):
        self.nc = bacc.Bacc(
            "TRN2", target_bir_lowering=False, debug=False, num_devices=8
        )
        self.consts = {}  # name -> np array (same for all cores)
        self.build()

    def inp(self, name, shape, dtype):
        return self.nc.declare_dram_parameter(name, list(shape), dtype, isOutput=False)

    def const(self, name, arr, dtype):
        h = self.nc.declare_dram_parameter(name, list(arr.shape), dtype, isOutput=False)
        self.consts[name] = arr
        return h

    def build(self):
        nc = self.nc
        x_in = self.inp("x", (6, N), FP32)
        self.out = nc.declare_dram_parameter("out", [2, 1], FP32, isOutput=True)

        wh = {}
        for li, (C, O) in enumerate(LAYERS):
            for nm, shape in [
                (f"wdh{li}", (C, O)),
                (f"wdl{li}", (C, O)),
                (f"cwh{li}", (C, O)),
                (f"cwl{li}", (C, O)),
            ]:
                wh[nm] = self.inp(nm, shape, BF16)
            for nm in ["a06", "b06", "a04", "b04"]:
                wh[f"{nm}_{li}"] = self.inp(f"{nm}_{li}", (O, 1), FP32)
        self.w5chunks = [(1, 0, 64), (2, 0, 64), (3, 0, 128), (4, 0, 128), (4, 128, 128)]
        for i, (fi, c0, cc) in enumerate(self.w5chunks):
            wh[f"w5h_{i}"] = self.inp(f"w5h_{i}", (cc, 1024), BF16)
            wh[f"w5l_{i}"] = self.inp(f"w5l_{i}", (cc, 1024), BF16)
        for nm in ["a06_5", "b06_5", "a04_5", "b04_5"]:
            wh[nm] = self.inp(nm, (1024, 1), FP32)
        wh["l1t"] = self.inp("l1t", (2048, 512), FP32)
        for nm in ["a06_l1", "b06_l1", "a04_l1", "b04_l1"]:
            wh[nm] = self.inp(nm, (512, 1), FP32)
        wh["l2t"] = self.inp("l2t", (512, 256), FP32)
        for nm in ["a06_l2", "b06_l2", "a04_l2", "b04_l2"]:
            wh[nm] = self.inp(nm, (256, 1), FP32)
        wh["l3t"] = self.inp("l3t", (256, 2), FP32)
        wh["l3b"] = self.inp("l3b", (2, 1), FP32)
        self.wh = wh

        self.dbg = {}
        if DEBUG:
            for li in range(4):
                self.dbg[f"srow{li}"] = nc.declare_dram_parameter(
                    f"dbg_srow{li}", [1, N], FP32, isOutput=True)
                self.dbg[f"glob{li}"] = nc.declare_dram_parameter(
                    f"dbg_glob{li}", [128, NTILES * K], U32, isOutput=True)
            self.dbg["x1h"] = nc.declare_dram_parameter(
                "dbg_x1h", [64, N], BF16, isOutput=True)
            self.dbg["x1l"] = nc.declare_dram_parameter(
                "dbg_x1l", [64, N], BF16, isOutput=True)
            self.dbg["x2h"] = nc.declare_dram_parameter(
                "dbg_x2h", [64, N], BF16, isOutput=True)
            self.dbg["x3h"] = nc.declare_dram_parameter(
                "dbg_x3h", [128, N], BF16, isOutput=True)
            self.dbg["x4h0"] = nc.declare_dram_parameter(
                "dbg_x4h0", [128, N], BF16, isOutput=True)
            self.dbg["x4h1"] = nc.declare_dram_parameter(
                "dbg_x4h1", [128, N], BF16, isOutput=True)
            self.dbg["pooled"] = nc.declare_dram_parameter(
                "dbg_pooled", [128, 16], FP32, isOutput=True)

        # packed-position pattern: column j (within a 1024-col chunk) -> j % 256
        posc = np.broadcast_to(
            (np.arange(1024, dtype=np.uint32) % GS)[None, :], (128, 1024)
        ).copy()
        POSC = self.const("POSC", posc, U32)

        with tile.TileContext(nc) as tc:
            self._emit(tc, x_in, wh, POSC)
        nc.compile()

    # ------------------------------------------------------------------
    def _emit(self, tc, x_in, wh, POSC):
        nc = self.nc
        from contextlib import ExitStack

        with ExitStack() as ctx:
            pers = ctx.enter_context(tc.tile_pool(name="pers", bufs=1))
            x1h = pers.tile([64, N], BF16, name="x1h", tag="x1h")
            x1l = pers.tile([64, N], BF16, name="x1l", tag="x1l")
            x2h = pers.tile([64, N], BF16, name="x2h", tag="x2h")
            x2l = pers.tile([64, N], BF16, name="x2l", tag="x2l")
            x3h = pers.tile([128, N], BF16, name="x3h", tag="x3h")
            x3l = pers.tile([128, N], BF16, name="x3l", tag="x3l")
            x4h0 = pers.tile([128, N], BF16, name="x4h0", tag="x4h0")
            x4h1 = pers.tile([128, N], BF16, name="x4h1", tag="x4h1")
            x4l0 = pers.tile([128, N], BF16, name="x4l0", tag="x4l0")
            x4l1 = pers.tile([128, N], BF16, name="x4l1", tag="x4l1")
            feat = [
                None,  # x0 installed below (scoped pool)
                HLFeat(64, [(x1h, x1l, 0)]),
                HLFeat(64, [(x2h, x2l, 0)]),
                HLFeat(128, [(x3h, x3l, 0)]),
                HLFeat(256, [(x4h0, x4l0, 0), (x4h1, x4l1, 0)]),
            ]
            srow = pers.tile([1, N], FP32, name="srow", tag="srow")

            consts = ctx.enter_context(tc.tile_pool(name="consts", bufs=1))
            mhalfcol = consts.tile([128, 1], FP32, name="mhalfcol")
            nc.vector.memset(mhalfcol[:], -0.5)
            onesrow = consts.tile([1, 128], BF16, name="onesrow")
            nc.vector.memset(onesrow[:], 1.0)
            msk = consts.tile([128, 1], U32, name="msk")
            nc.vector.memset(msk[:], 0xFFFFFF00)

            # edge-layer weights
            wpool = ctx.enter_context(tc.tile_pool(name="wpool", bufs=1))
            wt = {}
            for li, (C, O) in enumerate(LAYERS):
                for nm in [f"wdh{li}", f"wdl{li}", f"cwh{li}", f"cwl{li}"]:
                    wt[nm] = wpool.tile(list(wh[nm].shape), BF16, name=nm, tag=nm)
                    nc.sync.dma_start(wt[nm][:], wh[nm][:], single_packet=True)
                for nm in [f"a06_{li}", f"b06_{li}", f"a04_{li}", f"b04_{li}"]:
                    wt[nm] = [
                        wpool.tile(
                            [min(128, O - 128 * i), 1], FP32,
                            name=f"{nm}_o{i}", tag=f"{nm}_o{i}",
                        )
                        for i in range((O + 127) // 128)
                    ]
                    for i in range((O + 127) // 128):
                        nc.sync.dma_start(
                            wt[nm][i][:], wh[nm][128 * i : min(128 * (i + 1), O), :],
                            single_packet=True,
                        )

            lay_ctx = ExitStack()
            laypool = lay_ctx.enter_context(tc.tile_pool(name="laypool", bufs=1))
            srow_h = laypool.tile([1, N], BF16, name="srow_h", tag="srow_h")
            srow_l = laypool.tile([1, N], BF16, name="srow_l", tag="srow_l")
            post = laypool.tile([128, 1024], U32, name="post")
            nc.sync.dma_start(post[:], POSC[:], single_packet=True)

            xp_ctx = ExitStack()
            xp = xp_ctx.enter_context(tc.tile_pool(name="x0p", bufs=1))
            x0h = xp.tile([6, N], BF16, name="x0h", tag="x0h")
            x0l = xp.tile([6, N], BF16, name="x0l", tag="x0l")
            feat[0] = HLFeat(6, [(x0h, x0l, 0)])
            with tc.tile_pool(name="xin", bufs=1) as xin_pool:
                x0 = xin_pool.tile([6, N], FP32, name="x0")
                nc.sync.dma_start(x0[:], x_in[:], single_packet=True)
                xh32 = xin_pool.tile([6, N], FP32, name="xh32")
                nc.scalar.copy(feat[0].hi(0, 6), x0[:])  # cast fp32->bf16
                nc.scalar.copy(xh32[:], feat[0].hi(0, 6))  # back to fp32
                nc.vector.tensor_sub(feat[0].lo(0, 6), x0[:], xh32[:])

            for li, (C, O) in enumerate(LAYERS):
                self._edge_layer(
                    tc, li, C, O, feat[li], feat[li + 1], srow, srow_h, srow_l, wt,
                    post, mhalfcol, onesrow, msk,
                )
                if li == 0:
                    xp_ctx.close()
            if DEBUG:
                nc.sync.dma_start(self.dbg["x1h"][:], x1h[:], single_packet=True)
                nc.sync.dma_start(self.dbg["x1l"][:], x1l[:], single_packet=True)
                nc.sync.dma_start(self.dbg["x2h"][:], x2h[:], single_packet=True)
                nc.sync.dma_start(self.dbg["x3h"][:], x3h[:], single_packet=True)
                nc.sync.dma_start(self.dbg["x4h0"][:], x4h0[:], single_packet=True)
                nc.sync.dma_start(self.dbg["x4h1"][:], x4h1[:], single_packet=True)
            lay_ctx.close()

            self._head(tc, feat, wh)

    # ------------------------------------------------------------------
    def _edge_layer(
        self, tc, li, C, O, fin, fout, srow, srow_h, srow_l, wt, post,
        mhalfcol, onesrow, msk,
    ):
        nc = self.nc
        from contextlib import ExitStack

        nct = (C + 127) // 128
        not_ = (O + 127) // 128

        def cr(ci):
            c0 = 128 * ci
            return c0, min(128 * (ci + 1), C)

        with ExitStack() as octx:
          tpool = octx.enter_context(tc.tile_pool(name=f"tT{li}", bufs=1))
          with ExitStack() as lctx:
            # ---- layer setup: s row, sbig (= -s/2 broadcast), and tT ----
            sp = lctx.enter_context(tc.tile_pool(name=f"sq{li}", bufs=1))
            spp = lctx.enter_context(tc.tile_pool(name=f"spp{li}", bufs=2, space="PSUM"))
            xsq_tiles = []
            for ci in range(nct):
                c0, c1 = cr(ci)
                cc = c1 - c0
                xsq = sp.tile([cc, N], FP32, name="xsq", tag=f"xsq{ci}")
                # xsq = (hi + 2*lo) * hi, computed in place to save a tile
                nc.vector.tensor_scalar_mul(xsq[:], fin.lo(c0, c1), 2.0)
                nc.vector.tensor_add(xsq[:], xsq[:], fin.hi(c0, c1))
                nc.vector.tensor_mul(xsq[:], xsq[:], fin.hi(c0, c1))
                xsq_tiles.append((xsq, cc))
            for ch in range(N // 512):
                ps = spp.tile([1, 512], FP32, name="spsum", tag="spsum")
                for ti, (xsq, cc) in enumerate(xsq_tiles):
                    nc.tensor.matmul(
                        ps[:],
                        mhalfcol[0:cc, :],
                        xsq[:, 512 * ch : 512 * (ch + 1)],
                        start=(ti == 0),
                        stop=(ti == len(xsq_tiles) - 1),
                    )
                nc.scalar.copy(srow[:, 512 * ch : 512 * (ch + 1)], ps[:])

            # tT (O, N) = Wd @ x  (hi/lo 3-pass)
            tT = [
                tpool.tile([min(128, O - 128 * oi), N], FP32, name=f"tT{oi}", tag=f"tT{oi}")
                for oi in range(not_)
            ]
            tpp = lctx.enter_context(tc.tile_pool(name=f"tpp{li}", bufs=2, space="PSUM"))
            for oi in range(not_):
                o0, o1 = 128 * oi, min(128 * (oi + 1), O)
                for ch in range(N // 512):
                    pt = tpp.tile([o1 - o0, 512], FP32, name="tpsum", tag="tpsum")
                    sl = slice(512 * ch, 512 * (ch + 1))
                    mms = []
                    for ci in range(nct):
                        c0, c1 = cr(ci)
                        wdh = wt[f"wdh{li}"][c0:c1, o0:o1]
                        wdl = wt[f"wdl{li}"][c0:c1, o0:o1]
                        mms += [
                            (wdh, fin.hi(c0, c1, sl)),
                            (wdl, fin.hi(c0, c1, sl)),
                            (wdh, fin.lo(c0, c1, sl)),
                        ]
                    for mi, (lhsT, rhs) in enumerate(mms):
                        nc.tensor.matmul(
                            pt[:], lhsT, rhs,
                            start=(mi == 0), stop=(mi == len(mms) - 1),
                        )
                    nc.scalar.copy(tT[oi][:, sl], pt[:])

            # srow (= -s/2) hi/lo bf16 split for the in-group rank-1 matmuls
            srow_t = sp.tile([1, N], FP32, name="srow_t", tag="srow_t")
            nc.scalar.copy(srow_h[:], srow[:])
            nc.scalar.copy(srow_t[:], srow_h[:])
            nc.vector.tensor_sub(srow_l[:], srow[:], srow_t[:])
            if DEBUG:
                nc.sync.dma_start(self.dbg[f"srow{li}"][:], srow[:], single_packet=True)

          # ---- per-tile selection + gather loop ----
          with ExitStack() as tctx:
              dpp = tctx.enter_context(tc.tile_pool(name=f"dp{li}", bufs=2, space="PSUM"))
              cpp = tctx.enter_context(tc.tile_pool(name=f"cp{li}", bufs=2, space="PSUM"))
              sel = tctx.enter_context(tc.tile_pool(name=f"sel{li}", bufs=2))
              selp = tctx.enter_context(tc.tile_pool(name=f"selp{li}", bufs=3))
              gat = tctx.enter_context(tc.tile_pool(name=f"gat{li}", bufs=2))

              for ti in range(NTILES):
                  tsl = slice(128 * ti, 128 * (ti + 1))
                  cand = sel.tile([128, NCAND], FP32, name="cand", tag="cand")
                  for q in range(4):
                      pq = dpp.tile([128, 1024], FP32, name="dq", tag="dq")
                      for ch in range(2):
                          qsl = slice(1024 * q + 512 * ch, 1024 * q + 512 * (ch + 1))
                          psl = slice(512 * ch, 512 * (ch + 1))
                          mms = [
                              (onesrow[:], srow_h[:, qsl]),
                              (onesrow[:], srow_l[:, qsl]),
                          ]
                          for ci in range(nct):
                              c0, c1 = cr(ci)
                              mms += [
                                  (fin.hi(c0, c1, tsl), fin.hi(c0, c1, qsl)),
                                  (fin.lo(c0, c1, tsl), fin.hi(c0, c1, qsl)),
                                  (fin.hi(c0, c1, tsl), fin.lo(c0, c1, qsl)),
                              ]
                          for mi, (lhsT, rhs) in enumerate(mms):
                              nc.tensor.matmul(
                                  pq[:, psl], lhsT, rhs,
                                  start=(mi == 0), stop=(mi == len(mms) - 1),
                              )
                      # fused pack: (d & 0xFFFFFF00) | (j % 256), PSUM -> SBUF
                      pk = selp.tile([128, 1024], U32, name="pk", tag="pk")
                      nc.vector.scalar_tensor_tensor(
                          pk[:], pq[:].bitcast(U32), msk[:, 0:1], post[:],
                          op0=ALU.bitwise_and, op1=ALU.bitwise_or,
                      )
                      # stage A on this chunk (4 groups of 256)
                      for g in range(4):
                          gg = 4 * q + g
                          osl = slice(8 * gg, 8 * gg + 8)
                          gsl = slice(GS * g, GS * (g + 1))
                          nc.vector.max(cand[:, osl], pk[:, gsl].bitcast(FP32))
                  # stage B: top-24 of the 128 packed candidates
                  m24 = sel.tile([128, 24], FP32, name="m24", tag="m24")
                  posc = sel.tile([128, 24], U32, name="posc", tag="posc")
                  for r in range(3):
                      rsl = slice(8 * r, 8 * r + 8)
                      nc.vector.max(m24[:, rsl], cand[:])
                      nc.vector.max_index(posc[:, rsl], m24[:, rsl], cand[:])
                      if r < 2:
                          nc.vector.match_replace(cand[:], m24[:, rsl], cand[:], NEG_BIG)
                  # global idx = (posc//8)*256 | (packed & 0xFF)
                  gbase = sel.tile([128, K], U32, name="gbase", tag="gbase")
                  nc.vector.tensor_scalar(
                      gbase[:], posc[:, 0:K], 0x78, 5,
                      op0=ALU.bitwise_and, op1=ALU.logical_shift_left,
                  )
                  pos8 = sel.tile([128, K], U32, name="pos8", tag="pos8")
                  nc.vector.tensor_scalar(
                      pos8[:], m24[:, 0:K].bitcast(U32), 0xFF, None,
                      op0=ALU.bitwise_and,
                  )
                  glob = sel.tile([128, K], U32, name="glob", tag="glob")
                  nc.vector.tensor_tensor(
                      glob[:], gbase[:], pos8[:], op=ALU.bitwise_or
                  )
                  if DEBUG:
                      nc.sync.dma_start(
                          self.dbg[f"glob{li}"][:, K * ti : K * (ti + 1)],
                          glob[:], single_packet=True,
                      )
                  # widx[q, 20c+s] = glob16[16c+q, s], replicated to all 8
                  # 16-partition groups (ap_gather reads idxs per core group)
                  widx = sel.tile([128, 8 * K], U16, name="widx", tag="widx", bufs=3)
                  gv = glob[:].bitcast(U16)[:, 0 : 2 * K : 2]
                  for c in range(8):
                      nc.sync.dma_start(
                          widx[0:16, K * c : K * (c + 1)],
                          gv[16 * c : 16 * c + 16, :],
                          single_packet=True,
                      )
                  for gq in range(1, 8):
                      nc.sync.dma_start(
                          widx[16 * gq : 16 * gq + 16, :], widx[0:16, :],
                          single_packet=True,
                      )

                  for oi in range(not_):
                      o0, o1 = 128 * oi, min(128 * (oi + 1), O)
                      oo = o1 - o0
                      G = gat.tile([oo, 128 * K], FP32, name="G", tag="G")
                      nc.gpsimd.ap_gather(
                          G[:],
                          tT[oi][:],
                          widx[0:oo, :].bitcast(mybir.dt.int16),
                          channels=oo,
                          num_elems=N,
                          d=1,
                          num_idxs=128 * K,
                      )
                      M = gat.tile([oo, 128], FP32, name="M", tag=f"M{oi}")
                      nc.vector.tensor_reduce(
                          M[:],
                          G[:].rearrange("p (c s q) -> p c q s", c=8, s=K, q=16),
                          axis=AX.X,
                          op=ALU.max,
                      )
                      pc = cpp.tile([oo, 128], FP32, name="cps", tag=f"cps{oi}")
                      mms = []
                      for ci in range(nct):
                          c0, c1 = cr(ci)
                          cwh = wt[f"cwh{li}"][c0:c1, o0:o1]
                          cwl = wt[f"cwl{li}"][c0:c1, o0:o1]
                          mms += [
                              (cwh, fin.hi(c0, c1, tsl)),
                              (cwl, fin.hi(c0, c1, tsl)),
                              (cwh, fin.lo(c0, c1, tsl)),
                          ]
                      for mi, (lhsT, rhs) in enumerate(mms):
                          nc.tensor.matmul(
                              pc[:], lhsT, rhs,
                              start=(mi == 0), stop=(mi == len(mms) - 1),
                          )
                      nc.vector.tensor_add(M[:], M[:], pc[:])
                      z6 = gat.tile([oo, 128], FP32, name="z6", tag=f"z6_{oi}")
                      nc.scalar.activation(
                          z6[:], M[:], AF.Identity,
                          bias=wt[f"b06_{li}"][oi][:], scale=wt[f"a06_{li}"][oi][:],
                      )
                      z4 = gat.tile([oo, 128], FP32, name="z4", tag=f"z4_{oi}")
                      nc.scalar.activation(
                          z4[:], M[:], AF.Abs,
                          bias=wt[f"b04_{li}"][oi][:], scale=wt[f"a04_{li}"][oi][:],
                      )
                      zf = gat.tile([oo, 128], FP32, name="zf", tag=f"zf{oi}")
                      nc.vector.tensor_add(zf[:], z6[:], z4[:])
                      nc.scalar.copy(fout.hi(o0, o1, tsl), zf[:])  # hi (bf16)
                      zh32 = gat.tile([oo, 128], FP32, name="zh32", tag=f"zh32_{oi}")
                      nc.scalar.copy(zh32[:], fout.hi(o0, o1, tsl))
                      nc.vector.tensor_sub(fout.lo(o0, o1, tsl), zf[:], zh32[:])

    # ------------------------------------------------------------------
    def _head(self, tc, feat, wh):
        nc = self.nc
        from contextlib import ExitStack

        with ExitStack() as hctx:
            wp = hctx.enter_context(tc.tile_pool(name="headw", bufs=1))
            w5h, w5l = [], []
            for i, (fi, c0, cc) in enumerate(self.w5chunks):
                th = wp.tile([cc, 1024], BF16, name=f"w5h{i}", tag=f"w5h{i}")
                nc.sync.dma_start(th[:], wh[f"w5h_{i}"][:], single_packet=True)
                w5h.append(th)
                tl = wp.tile([cc, 1024], BF16, name=f"w5l{i}", tag=f"w5l{i}")
                nc.sync.dma_start(tl[:], wh[f"w5l_{i}"][:], single_packet=True)
                w5l.append(tl)
            prm = {}
            for nm in [
                "a06_5", "b06_5", "a04_5", "b04_5",
                "a06_l1", "b06_l1", "a04_l1", "b04_l1",
                "a06_l2", "b06_l2", "a04_l2", "b04_l2",
                "l3b",
            ]:
                shp = list(wh[nm].shape)
                if shp[0] > 128:
                    prm[nm] = [
                        wp.tile([128, 1], FP32, name=f"{nm}_{i}", tag=f"{nm}_{i}")
                        for i in range(shp[0] // 128)
                    ]
                    for i in range(shp[0] // 128):
                        nc.sync.dma_start(
                            prm[nm][i][:], wh[nm][128 * i : 128 * (i + 1), :],
                            single_packet=True,
                        )
                else:
                    prm[nm] = wp.tile(shp, FP32, name=nm, tag=nm)
                    nc.sync.dma_start(prm[nm][:], wh[nm][:], single_packet=True)
            l1t = [wp.tile([128, 512], FP32, name=f"l1t{i}", tag=f"l1t{i}") for i in range(16)]
            for i in range(16):
                nc.sync.dma_start(l1t[i][:], wh["l1t"][128 * i : 128 * (i + 1), :], single_packet=True)
            l2t = [wp.tile([128, 256], FP32, name=f"l2t{i}", tag=f"l2t{i}") for i in range(4)]
            for i in range(4):
                nc.sync.dma_start(l2t[i][:], wh["l2t"][128 * i : 128 * (i + 1), :], single_packet=True)
            l3t = [wp.tile([128, 2], FP32, name=f"l3t{i}", tag=f"l3t{i}") for i in range(2)]
            for i in range(2):
                nc.sync.dma_start(l3t[i][:], wh["l3t"][128 * i : 128 * (i + 1), :], single_packet=True)

            kchunks = [
                (feat[fi], c0, cc, i) for i, (fi, c0, cc) in enumerate(self.w5chunks)
            ]

            pool5 = hctx.enter_context(tc.tile_pool(name="pool5", bufs=2))
            psum5 = hctx.enter_context(tc.tile_pool(name="psum5", bufs=2, space="PSUM"))
            pooled = wp.tile([128, 16], FP32, name="pooled", tag="pooled")
            for oi in range(8):
                o0 = 128 * oi
                h = pool5.tile([128, N], FP32, name="h5", tag="h5")
                for ch in range(N // 512):
                    sl = slice(512 * ch, 512 * (ch + 1))
                    pt = psum5.tile([128, 512], FP32, name="p5", tag="p5")
                    mms = []
                    for (fbuf, c0, cc, wi) in kchunks:
                        whk = w5h[wi][:, o0 : o0 + 128]
                        wlk = w5l[wi][:, o0 : o0 + 128]
                        xhc = fbuf.hi(c0, c0 + cc, sl)
                        xlc = fbuf.lo(c0, c0 + cc, sl)
                        mms += [(whk, xhc), (wlk, xhc), (whk, xlc)]
                    for mi, (lhsT, rhs) in enumerate(mms):
                        nc.tensor.matmul(
                            pt[:], lhsT, rhs,
                            start=(mi == 0), stop=(mi == len(mms) - 1),
                        )
                    z6 = pool5.tile([128, 512], FP32, name="z65", tag="z65")
                    nc.scalar.activation(
                        z6[:], pt[:], AF.Identity,
                        bias=prm["b06_5"][oi][:], scale=prm["a06_5"][oi][:],
                    )
                    z4 = pool5.tile([128, 512], FP32, name="z45", tag="z45")
                    nc.scalar.activation(
                        z4[:], pt[:], AF.Abs,
                        bias=prm["b04_5"][oi][:], scale=prm["a04_5"][oi][:],
                    )
                    nc.vector.tensor_add(h[:, sl], z6[:], z4[:])
                nc.vector.tensor_reduce(
                    pooled[:, oi : oi + 1], h[:], axis=AX.X, op=ALU.max
                )
                nc.vector.tensor_reduce(
                    pooled[:, 8 + oi : 8 + oi + 1], h[:], axis=AX.X, op=ALU.add
                )
            if DEBUG:
                nc.sync.dma_start(self.dbg["pooled"][:], pooled[:], single_packet=True)

            # ---- MLP head ----
            hp = hctx.enter_context(tc.tile_pool(name="hp", bufs=1))
            hpp = hctx.enter_context(tc.tile_pool(name="hpp", bufs=1, space="PSUM"))
            o1 = hp.tile([128, 4], FP32, name="o1", tag="o1")
            for oi in range(4):
                p1 = hpp.tile([128, 1], FP32, name="p1", tag="p1")
                for ki in range(16):
                    nc.tensor.matmul(
                        p1[:],
                        l1t[ki][:, 128 * oi : 128 * (oi + 1)],
                        pooled[:, ki : ki + 1],
                        start=(ki == 0),
                        stop=(ki == 15),
                    )
                z6 = hp.tile([128, 1], FP32, name="hz6", tag="hz6")
                nc.scalar.activation(
                    z6[:], p1[:], AF.Identity,
                    bias=prm["b06_l1"][oi][:], scale=prm["a06_l1"][oi][:],
                )
                z4 = hp.tile([128, 1], FP32, name="hz4", tag="hz4")
                nc.scalar.activation(
                    z4[:], p1[:], AF.Abs,
                    bias=prm["b04_l1"][oi][:], scale=prm["a04_l1"][oi][:],
                )
                nc.vector.tensor_add(o1[:, oi : oi + 1], z6[:], z4[:])
            o2 = hp.tile([128, 2], FP32, name="o2", tag="o2")
            for oi in range(2):
                p2 = hpp.tile([128, 1], FP32, name="p2", tag="p2")
                for ki in range(4):
                    nc.tensor.matmul(
                        p2[:],
                        l2t[ki][:, 128 * oi : 128 * (oi + 1)],
                        o1[:, ki : ki + 1],
                        start=(ki == 0),
                        stop=(ki == 3),
                    )
                z6 = hp.tile([128, 1], FP32, name="hz6b", tag="hz6b")
                nc.scalar.activation(
                    z6[:], p2[:], AF.Identity,
                    bias=prm["b06_l2"][oi][:], scale=prm["a06_l2"][oi][:],
                )
                z4 = hp.tile([128, 1], FP32, name="hz4b", tag="hz4b")
                nc.scalar.activation(
                    z4[:], p2[:], AF.Abs,
                    bias=prm["b04_l2"][oi][:], scale=prm["a04_l2"][oi][:],
                )
                nc.vector.tensor_add(o2[:, oi : oi + 1], z6[:], z4[:])
            p3 = hpp.tile([2, 1], FP32, name="p3", tag="p3")
            for ki in range(2):
                nc.tensor.matmul(
                    p3[:],
                    l3t[ki][:],
                    o2[:, ki : ki + 1],
                    start=(ki == 0),
                    stop=(ki == 1),
                )
            ofin = hp.tile([2, 1], FP32, name="ofin", tag="ofin")
            nc.scalar.activation(ofin[:], p3[:], AF.Identity, bias=prm["l3b"][:], scale=1.0)
            nc.sync.dma_start(self.out[:], ofin[:], single_packet=True)


# ----------------------------------------------------------------------
_PROG = None


def _get_prog():
    global _PROG
    if _PROG is None:
        _PROG = Prog()
    return _PROG


def _host_params(inputs):
    d = {}
    for li in range(4):
        C, O = LAYERS[li]
        W = np.asarray(inputs[f"W{li+1}"], np.float32)
        Wd, Wc = W[:, :C], W[:, C:]
        wdh, wdl = _split_hl(np.ascontiguousarray(Wd.T))
        cwh, cwl = _split_hl(np.ascontiguousarray((Wc - Wd).T))
        d[f"wdh{li}"], d[f"wdl{li}"] = wdh, wdl
        d[f"cwh{li}"], d[f"cwl{li}"] = cwh, cwl
        a, b = _bn_affine(
            np.asarray(inputs[f"g{li+1}"]), np.asarray(inputs[f"b{li+1}"]),
            np.asarray(inputs[f"m{li+1}"]), np.asarray(inputs[f"v{li+1}"]),
        )
        d[f"a06_{li}"] = np.ascontiguousarray((0.6 * a)[:, None])
        d[f"b06_{li}"] = np.ascontiguousarray((0.6 * b)[:, None])
        d[f"a04_{li}"] = np.ascontiguousarray((0.4 * a)[:, None])
        d[f"b04_{li}"] = np.ascontiguousarray((0.4 * b)[:, None])
    w5t = np.ascontiguousarray(np.asarray(inputs["W5"], np.float32).T)  # (512,1024)
    w5chunks = [64, 64, 128, 128, 128]
    k0 = 0
    for i, cc in enumerate(w5chunks):
        h, l = _split_hl(np.ascontiguousarray(w5t[k0 : k0 + cc]))
        d[f"w5h_{i}"], d[f"w5l_{i}"] = h, l
        k0 += cc
    a5, b5 = _bn_affine(
        np.asarray(inputs["g5"]), np.asarray(inputs["b5"]),
        np.asarray(inputs["m5"]), np.asarray(inputs["v5"]),
    )
    d["a06_5"] = np.ascontiguousarray((0.6 * a5)[:, None])
    d["b06_5"] = np.ascontiguousarray((0.6 * b5)[:, None])
    d["a04_5"] = np.ascontiguousarray((0.4 * a5)[:, None])
    d["b04_5"] = np.ascontiguousarray((0.4 * b5)[:, None])
    L1 = np.asarray(inputs["L1"], np.float32).copy()
    L1[:, 1024:] *= 1.0 / N
    d["l1t"] = np.ascontiguousarray(L1.T)
    al1, bl1 = _bn_affine(
        np.asarray(inputs["gl1"]), np.asarray(inputs["bl1"]),
        np.asarray(inputs["ml1"]), np.asarray(inputs["vl1"]),
    )
    d["a06_l1"] = np.ascontiguousarray((0.6 * al1)[:, None])
    d["b06_l1"] = np.ascontiguousarray((0.6 * bl1)[:, None])
    d["a04_l1"] = np.ascontiguousarray((0.4 * al1)[:, None])
    d["b04_l1"] = np.ascontiguousarray((0.4 * bl1)[:, None])
    L2 = np.asarray(inputs["L2"], np.float32)
    d["l2t"] = np.ascontiguousarray(L2.T)
    al2, bl2_ = _bn_affine(
        np.asarray(inputs["gl2"]), np.asarray(inputs["bl2"]),
        np.asarray(inputs["ml2"]), np.asarray(inputs["vl2"]),
    )
    bl2 = al2 * np.asarray(inputs["L2b"], np.float32) + bl2_
    d["a06_l2"] = np.ascontiguousarray((0.6 * al2)[:, None])
    d["b06_l2"] = np.ascontiguousarray((0.6 * bl2)[:, None])
    d["a04_l2"] = np.ascontiguousarray((0.4 * al2)[:, None])
    d["b04_l2"] = np.ascontiguousarray((0.4 * bl2)[:, None])
    d["l3t"] = np.ascontiguousarray(np.asarray(inputs["L3"], np.float32).T)
    d["l3b"] = np.ascontiguousarray(np.asarray(inputs["L3b"], np.float32)[:, None])
    return d


def _in_maps(prog, inputs):
    params = _host_params(inputs)
    x = np.asarray(inputs["x"], np.float32)
    in_maps = []
    for b in range(x.shape[0]):
        m = {"x": np.ascontiguousarray(x[b])}
        m.update(params)
        m.update(prog.consts)
        in_maps.append(m)
    return in_maps


def _post(res):
    out = np.stack(
        [np.asarray(res.results[b]["out"]).reshape(2) for b in range(8)]
    )
    return out.astype(np.float32)


def kernel(**inputs):
    prog = _get_prog()
    res = run_bass_kernel_spmd(prog.nc, _in_maps(prog, inputs), list(range(8)))
    return _post(res)


if __name__ == "__main__":
    import reference as ref

    inputs = {k: np.asarray(v) for k, v in ref.setup_inputs().items()}
    got = kernel(**inputs)
    exp = np.asarray(ref.reference(**inputs))
    err = np.abs(got - exp).max() / np.abs(exp).max()
    print("expected:\n", exp)
    print("got:\n", got)
    print("rel err:", err)
